# revision 4
# baseline (speedup 1.0000x reference)
"""Trainium2 Bass kernel v3: fused attention block.

Sharding: 2-way batch x 4-way head-group over 8 cores; host sums the 4
head-group partials per batch element (f32).

Per-core pipeline (1 batch elem, NH=4 heads, S=2048, DM=2048, HD=128):
  QKV proj:   error-compensated fp8 DoubleRow matmuls:
                x@(32w) ~= xh@wh + xl@wh + xh@wl  (xh=fp8(x), xl=fp8(x-xh),
                wh=fp8(32w), wl=fp8(32w-wh); the 32x pre-scale keeps w and
                its residual out of fp8-subnormal range, and the per-head
                RMSNorm cancels the scale exactly)
              0.5 cycles/row with 256-deep contraction: ~25% cheaper than
              bf16 and slightly more accurate.
  RMSNorm:    per-tensor (q,k,v) chains to keep PSUM turnover fast:
              ACT square -> DVE reduce (bf16) -> ACT sqrt -> DVE recip;
              normalization fused into PSUM evacuation via stride-0
              broadcast multiply on DVE.
  RoPE:       bf16 on DVE (2x); q,k transposed via DMA-transpose
  scores:     bf16 matmuls; exp on ACT -> bf16 P^T; causal mask applied
              multiplicatively on P^T by GpSimd (SBUF-only engine)
  denom:      bf16 ones-matmul [128,128] -> PSUM rows are the denominator
              broadcast across all partitions
  PV, o_proj: bf16 matmuls; y emitted bf16 (host accumulates in f32)
"""

import numpy as np
import ml_dtypes

import concourse.bass as bass
import concourse.mybir as mybir
import concourse.tile as tile
from concourse.bass_utils import run_bass_kernel_spmd

F8 = mybir.dt.float8e4
BF16 = mybir.dt.bfloat16
F32 = mybir.dt.float32
DR = mybir.MatmulPerfMode.DoubleRow
AF = mybir.ActivationFunctionType
OP = mybir.AluOpType
AX = mybir.AxisListType

B, S, DM = 2, 2048, 2048
HD, H = 128, 16
NH = 4                    # heads per core
NW = 4                    # waves of 512 tokens
EPS = 1e-6
WSCALE = 32.0             # w_qkv pre-scale; cancelled by RMSNorm
SCALE = 1.0 / np.sqrt(HD)
EXP_BIAS = -0.7


def legalize_multi_waits(nc):
    """Walrus codegen only supports one sync wait per instruction; move
    extras onto preceding no-op carriers on the same engine."""
    n = 0
    for f in nc.m.functions:
        for blk in f.blocks:
            newlist = []
            for ins in blk.instructions:
                si = ins.sync_info
                if si is not None and len(si.on_wait) > 1:
                    waits = list(si.on_wait)
                    for i, w in enumerate(waits[:-1]):
                        nop = mybir.InstNoOp(
                            name=f"{ins.name}-wsplit{i}",
                            sync_info=mybir.SyncInfo(on_wait=[w], on_update=[]),
                            bass_nofuse=True,
                            engine=ins.engine,
                        )
                        newlist.append(nop)
                        n += 1
                    ins.sync_info = mybir.SyncInfo(
                        on_wait=[waits[-1]], on_update=list(si.on_update)
                    )
                newlist.append(ins)
            blk.instructions = newlist
    return n


def build_nc(repeat=1):
    nc = bass.Bass()
    xh = nc.declare_dram_parameter("xh", [128, 8, NW, 4, 2, 128], F8,
                                   isOutput=False)
    xl = nc.declare_dram_parameter("xl", [128, 8, NW, 4, 2, 128], F8,
                                   isOutput=False)
    wh = nc.declare_dram_parameter("wh", [128, 8, 3, 2, 512], F8,
                                   isOutput=False)
    wl = nc.declare_dram_parameter("wl", [128, 8, 3, 2, 512], F8,
                                   isOutput=False)
    woh = nc.declare_dram_parameter("woh", [128, 2, 4, 2, 512], F8,
                                    isOutput=False)
    wol = nc.declare_dram_parameter("wol", [128, 2, 4, 2, 512], F8,
                                    isOutput=False)
    cosR = nc.declare_dram_parameter("cosR", [S, NH * HD], BF16, isOutput=False)
    sinR = nc.declare_dram_parameter("sinR", [S, NH * HD], BF16, isOutput=False)
    maskM = nc.declare_dram_parameter("maskM", [128, 4, 512], F8,
                                      isOutput=False)
    maskA = nc.declare_dram_parameter("maskA", [128, 4, 512], F8,
                                      isOutput=False)
    y = nc.declare_dram_parameter("y", [S, DM], BF16, isOutput=True)

    with tile.TileContext(nc) as tc:
        with tc.tile_pool(name="glob", bufs=1) as glob, \
             tc.tile_pool(name="xsp", bufs=2) as xsp, \
             tc.tile_pool(name="sc1", bufs=3) as sc1, \
             tc.tile_pool(name="rop", bufs=2) as rop, \
             tc.tile_pool(name="qtp", bufs=2) as qtp, \
             tc.tile_pool(name="otp", bufs=2) as otp, \
             tc.tile_pool(name="ptp", bufs=6) as ptp, \
             tc.tile_pool(name="rcpp", bufs=2) as rcpp, \
             tc.tile_pool(name="ysp", bufs=8) as ysp, \
             tc.tile_pool(name="p1", bufs=3, space="PSUM") as p1, \
             tc.tile_pool(name="ps2", bufs=3, space="PSUM") as ps2, \
             tc.tile_pool(name="pod", bufs=2, space="PSUM") as pod:
            wht = glob.tile([128, 8, 3, 2, 512], F8, tag="wh", name="wht")
            wlt = glob.tile([128, 8, 3, 2, 512], F8, tag="wl", name="wlt")
            woht = glob.tile([128, 2, 4, 2, 512], F8, tag="woh", name="woht")
            wolt = glob.tile([128, 2, 4, 2, 512], F8, tag="wol", name="wolt")
            kT = glob.tile([128, NH, S], BF16, tag="kT", name="kT")
            vsbh = glob.tile([128, NH, 16, 128], F8, tag="vsbh", name="vsbh")
            vsbl = glob.tile([128, NH, 16, 128], F8, tag="vsbl", name="vsbl")
            mask_sb = glob.tile([128, 4, 512], F8, tag="mask", name="mask_sb")
            maska_sb = glob.tile([128, 4, 512], F8, tag="maska",
                                 name="maska_sb")
            ones8 = glob.tile([128, 2, 128], F8, tag="ones8", name="ones8")
            eps_t = glob.tile([128, 1], F32, tag="eps", name="eps_t")
            ebias = glob.tile([128, 1], F32, tag="ebias", name="ebias")
            nc.vector.memset(ones8, 1.0)
            nc.vector.memset(eps_t, EPS)
            nc.vector.memset(ebias, EXP_BIAS)
            # first-wave-critical loads are interleaved per dm-pair inside
            # emit_proj_wave(0) so PE can start as soon as pair 0 lands
            deferred = [lambda: nc.sync.dma_start(out=mask_sb, in_=maskM[:]),
                        lambda: nc.sync.dma_start(out=maska_sb, in_=maskA[:]),
                        lambda: nc.sync.dma_start(out=woht, in_=woh[:]),
                        lambda: nc.sync.dma_start(out=wolt, in_=wol[:])]

            def emit_proj_wave(rep, wave):
                xht = xsp.tile([128, 8, 4, 2, 128], F8, tag="xh", name="xht")
                xlt = xsp.tile([128, 8, 4, 2, 128], F8, tag="xl", name="xlt")
                if rep == 0 and wave == 0:
                    for dp in range(8):
                        nc.sync.dma_start(out=xht[:, dp], in_=xh[:, dp, 0])
                        nc.sync.dma_start(out=wht[:, dp], in_=wh[:, dp])
                    for dp in range(8):
                        nc.sync.dma_start(out=xlt[:, dp], in_=xl[:, dp, 0])
                    for dp in range(8):
                        nc.sync.dma_start(out=wlt[:, dp], in_=wl[:, dp])
                else:
                    nc.sync.dma_start(out=xht, in_=xh[:, :, wave])
                    nc.sync.dma_start(out=xlt, in_=xl[:, :, wave])
                while deferred:
                    deferred.pop(0)()
                qTw = qtp.tile([128, NH, 512], BF16, tag="qTw", name="qTw")
                yield qTw
                for tcl in range(4):
                    tcg = wave * 4 + tcl
                    pst = []
                    for blk in range(3):
                        ps = p1.tile([128, 512], F32, tag="p1",
                                     name=f"ps{'qkv'[blk]}")
                        pst.append(ps)
                        for dp in range(8):
                            nc.tensor.matmul(
                                ps, xht[:, dp, tcl], wht[:, dp, blk],
                                start=(dp == 0), stop=False, perf_mode=DR,
                                skip_group_check=True)
                        for dp in range(8):
                            nc.tensor.matmul(
                                ps, xlt[:, dp, tcl], wht[:, dp, blk],
                                start=False, stop=False, perf_mode=DR,
                                skip_group_check=True)
                        for dp in range(8):
                            nc.tensor.matmul(
                                ps, xht[:, dp, tcl], wlt[:, dp, blk],
                                start=False, stop=(dp == 7), perf_mode=DR,
                                skip_group_check=True)
                    # per-tensor RMSNorm chain + fused normalize-evacuate
                    rss = []
                    for blk in range(3):
                        ps4 = pst[blk].rearrange("p (h d) -> p h d", h=4)
                        sq = sc1.tile([128, 4, 128], BF16, tag=f"sq{blk}",
                                      name=f"sq{blk}")
                        nc.scalar.activation(sq, ps4, AF.Square)
                        ssq = sc1.tile([128, 4], BF16, tag=f"ssq{blk}",
                                       name=f"ssq{blk}")
                        with nc.allow_low_precision("rms stats tolerate bf16"):
                            nc.vector.tensor_reduce(out=ssq, in_=sq,
                                                    axis=AX.X, op=OP.add)
                        s1 = sc1.tile([128, 4], F32, tag=f"s1{blk}",
                                      name=f"s1{blk}")
                        nc.scalar.activation(s1, ssq, AF.Sqrt, bias=eps_t[:],
                                             scale=1.0 / HD)
                        rs = sc1.tile([128, 4], F32, tag=f"rs{blk}",
                                      name=f"rs{blk}")
                        nc.vector.reciprocal(rs, s1)
                        rss.append(rs)
                    qkn = sc1.tile([128, 8, 128], BF16, tag="qkn", name="qkn")
                    for blk in range(2):
                        nc.vector.tensor_tensor(
                            qkn[:, 4 * blk:4 * blk + 4],
                            pst[blk].rearrange("p (h d) -> p h d", h=4),
                            rss[blk][:, :, None].broadcast_to([128, 4, 128]),
                            OP.mult)
                    vbuf = sc1.tile([128, 4, 128], BF16, tag="vbuf",
                                    name="vbuf")
                    nc.vector.tensor_tensor(
                        vbuf, pst[2].rearrange("p (h d) -> p h d", h=4),
                        rss[2][:, :, None].broadcast_to([128, 4, 128]),
                        OP.mult)
                    nc.vector.tensor_copy(vsbh[:, :, tcg, :], vbuf)
                    nc.vector.tensor_tensor(vsbl[:, :, tcg, :], vbuf,
                                            vsbh[:, :, tcg, :], OP.subtract)
                    # RoPE on DVE (bf16)
                    cos_t = rop.tile([128, NH, 128], BF16, tag="cos",
                                     name="cos_t")
                    sin_t = rop.tile([128, NH, 128], BF16, tag="sin",
                                     name="sin_t")
                    nc.sync.dma_start(
                        out=cos_t,
                        in_=cosR[tcg * 128:(tcg + 1) * 128]
                        .rearrange("p (h d) -> p h d", h=NH))
                    nc.sync.dma_start(
                        out=sin_t,
                        in_=sinR[tcg * 128:(tcg + 1) * 128]
                        .rearrange("p (h d) -> p h d", h=NH))
                    for qk in range(2):
                        src = qkn[:, 4 * qk:4 * qk + 4]
                        ct = rop.tile([128, NH, 128], BF16, tag=f"ct{qk}",
                                      name=f"ct{qk}")
                        nc.vector.tensor_tensor(ct, src, cos_t, OP.mult)
                        tt = rop.tile([128, NH, 128], BF16, tag=f"tt{qk}",
                                      name=f"tt{qk}")
                        nc.vector.tensor_tensor(
                            tt[:, :, 0:64], src[:, :, 64:128],
                            sin_t[:, :, 0:64], OP.mult)
                        nc.vector.tensor_tensor(
                            tt[:, :, 64:128], src[:, :, 0:64],
                            sin_t[:, :, 64:128], OP.mult)
                        qr = rop.tile([128, NH, 128], BF16, tag=f"qr{qk}",
                                      name=f"qr{qk}")
                        nc.vector.tensor_tensor(qr, ct, tt, OP.add)
                        if qk == 0:
                            dst = qTw[:, :, tcl * 128:(tcl + 1) * 128]
                        else:
                            dst = kT[:, :, tcg * 128:(tcg + 1) * 128]
                        nc.sync.dma_start_transpose(dst, qr[:, :, :])
                    yield tcl

            def pv_pair(PT, h, kt0, psO, psD, is_first, is_last):
                nc.tensor.matmul(psO, vsbh[:, h, kt0:kt0 + 2, :], PT[:],
                                 start=is_first, stop=False, perf_mode=DR,
                                 skip_group_check=True)
                nc.tensor.matmul(psO, vsbl[:, h, kt0:kt0 + 2, :], PT[:],
                                 start=False, stop=is_last, perf_mode=DR,
                                 skip_group_check=True)
                nc.tensor.matmul(psD, ones8[:], PT[:],
                                 start=is_first, stop=is_last, perf_mode=DR,
                                 skip_group_check=True)

            def emit_attn_oproj(wave, qTw):
                nkt = 4 * (wave + 1)
                oTwh = otp.tile([128, 4, NH, 128], F8, tag="oTwh",
                                name="oTwh")
                oTwl = otp.tile([128, 4, NH, 128], F8, tag="oTwl",
                                name="oTwl")
                for h in range(NH):
                    psO = pod.tile([128, 512], F32, tag="pod", name="psO")
                    psD = pod.tile([128, 512], F32, tag="pod", name="psD")
                    pend = []
                    PT = None
                    npair = nkt // 2
                    for kt in range(nkt):
                        if kt % 2 == 0:
                            PT = ptp.tile([128, 2, 512], F8, tag="PT",
                                          name="PT")
                        psS = ps2.tile([128, 512], F32, tag="ps2", name="psS")
                        nc.tensor.matmul(
                            psS, kT[:, h, kt * 128:(kt + 1) * 128],
                            qTw[:, h], start=True, stop=True)
                        jj = kt - 4 * wave
                        late = jj >= 0
                        if late:
                            nc.vector.tensor_tensor(psS, psS, maska_sb[:, jj],
                                                    OP.add)
                        nc.scalar.activation(PT[:, kt % 2], psS, AF.Exp,
                                             bias=ebias[:], scale=SCALE)
                        if jj >= 0 and not late:
                            nc.gpsimd.tensor_tensor(
                                PT[:, kt % 2], PT[:, kt % 2], mask_sb[:, jj],
                                OP.mult)
                        if kt % 2 == 1:
                            pend.append((PT, h, kt - 1, psO, psD, kt == 1,
                                         kt == nkt - 1))
                            if len(pend) > 3:
                                pv_pair(*pend.pop(0))
                    for pp in pend:
                        pv_pair(*pp)
                    oU = rcpp.tile([128, 512], BF16, tag="oU", name="oU")
                    if wave >= 2:
                        nc.vector.tensor_copy(oU, psO)
                    else:
                        nc.scalar.copy(oU, psO)
                    rcp = rcpp.tile([128, 512], BF16, tag="rcp", name="rcp")
                    with nc.allow_low_precision("denom recip bf16"):
                        nc.vector.reciprocal(rcp, psD)
                    o_star = rcpp.tile([128, 4, 128], BF16, tag="ostar",
                                       name="o_star")
                    nc.vector.tensor_tensor(
                        o_star, oU.rearrange("p (a t) -> p a t", a=4),
                        rcp.rearrange("p (a t) -> p a t", a=4), OP.mult)
                    nc.vector.tensor_copy(oTwh[:, :, h, :], o_star)
                    nc.vector.tensor_tensor(oTwl[:, :, h, :], o_star,
                                            oTwh[:, :, h, :], OP.subtract)
                    yield h

                yield "heads-done"
                for tcl in range(4):
                    for dmb in range(4):
                        psY = ps2.tile([128, 512], F32, tag="ps2", name="psY")
                        for jp in range(2):
                            nc.tensor.matmul(
                                psY, oTwh[:, tcl, 2 * jp:2 * jp + 2, :],
                                woht[:, jp, dmb], start=(jp == 0), stop=False,
                                perf_mode=DR, skip_group_check=True)
                            nc.tensor.matmul(
                                psY, oTwl[:, tcl, 2 * jp:2 * jp + 2, :],
                                woht[:, jp, dmb], start=False, stop=False,
                                perf_mode=DR, skip_group_check=True)
                            nc.tensor.matmul(
                                psY, oTwh[:, tcl, 2 * jp:2 * jp + 2, :],
                                wolt[:, jp, dmb], start=False,
                                stop=(jp == 1), perf_mode=DR,
                                skip_group_check=True)
                        ysb = ysp.tile([128, 512], BF16, tag="ysb", name="ysb")
                        if (tcl + dmb) % 2 == 0:
                            nc.vector.tensor_copy(ysb, psY)
                        else:
                            nc.scalar.copy(ysb, psY)
                        nc.sync.dma_start(
                            out=y[(wave * 4 + tcl) * 128:
                                  (wave * 4 + tcl + 1) * 128,
                                  dmb * 512:(dmb + 1) * 512],
                            in_=ysb)
                    yield ("oproj", tcl)

            for rep in range(repeat):
                pending = None
                prev_attn = None
                for wave in range(NW):
                    gen_attn = emit_attn_oproj(*pending) if pending else None
                    gen_proj = emit_proj_wave(rep, wave)
                    qTw = next(gen_proj)
                    for tcl in range(4):
                        next(gen_proj, None)
                        if gen_attn is not None:
                            next(gen_attn, None)
                    if gen_attn is not None and wave < NW - 1:
                        for _ in gen_attn:
                            pass
                    pending = (wave, qTw)
                    prev_attn = gen_attn
                # final wave: interleave o_proj(NW-2) into attn(NW-1) heads
                gen_attn = emit_attn_oproj(*pending)
                for h in range(NH):
                    next(gen_attn, None)          # head h of attn(NW-1)
                    if prev_attn is not None:
                        next(prev_attn, None)     # one o_proj tcl of wave NW-2
                if prev_attn is not None:
                    for _ in prev_attn:
                        pass
                for _ in gen_attn:
                    pass

    return nc


_NC_CACHE = None
_NC_LEGALIZED = False


def get_nc(legalized=False):
    global _NC_CACHE, _NC_LEGALIZED
    if _NC_CACHE is None:
        _NC_CACHE = build_nc()
    if legalized and not _NC_LEGALIZED:
        legalize_multi_waits(_NC_CACHE)
        _NC_LEGALIZED = True
    return _NC_CACHE


def prep_core_inputs(hidden_states, cos, sin, w_qkv, w_o):
    """Build the 8 per-core input maps (host-side shard + layout + fp8)."""
    f8 = ml_dtypes.float8_e4m3
    bf = ml_dtypes.bfloat16
    f32 = np.float32
    hidden_states = np.asarray(hidden_states, dtype=f32)
    cos = np.asarray(cos, dtype=f32)
    sin = np.asarray(sin, dtype=f32)
    w_qkv = np.asarray(w_qkv, dtype=f32)
    w_o = np.asarray(w_o, dtype=f32)

    sinm = np.concatenate([-sin[:, :64], sin[:, 64:]], axis=1)
    cosR = np.ascontiguousarray(np.tile(cos, (1, NH))).astype(bf)
    sinR = np.ascontiguousarray(np.tile(sinm, (1, NH))).astype(bf)

    p = np.arange(128)[:, None]
    c = np.arange(512)[None, :]
    maskM = np.stack(
        [(p + 128 * j <= c).astype(np.float32) for j in range(4)],
        axis=1).astype(f8)                                   # [128, 4, 512]
    maskA = np.stack(
        [np.where(p + 128 * j <= c, 0.0, -240.0) for j in range(4)],
        axis=1).astype(f8)

    def pack_x(xT):
        # [DM, S] -> [128, 8, NW, 4, 2, 128]
        r = xT.reshape(8, 2, 128, NW, 4, 128)
        return np.ascontiguousarray(r.transpose(2, 0, 3, 4, 1, 5))

    def pack_w(wT):
        # [DM, 1536] -> [128, 8, 3, 2, 512]
        r = wT.reshape(8, 2, 128, 3, 512)
        return np.ascontiguousarray(r.transpose(2, 0, 3, 1, 4))

    in_maps = []
    for core in range(8):
        b, hg = core // 4, core % 4
        r0 = hg * NH * HD
        xT = np.ascontiguousarray(hidden_states[b].T)        # [DM, S]
        xTh = xT.astype(f8)
        xTl = (xT - xTh.astype(f32)).astype(f8)
        wrows = np.concatenate([
            w_qkv[r0:r0 + NH * HD],
            w_qkv[H * HD + r0:H * HD + r0 + NH * HD],
            w_qkv[2 * H * HD + r0:2 * H * HD + r0 + NH * HD]], axis=0)
        wT = np.ascontiguousarray(wrows.T) * WSCALE          # [DM, 1536]
        wTh = wT.astype(f8)
        wTl = (wT - wTh.astype(f32)).astype(f8)
        # wo [128, 2, 4, 2, 512]: wo_hg^T[(2j+i)*128+p, blk*512+c] * 32
        woT = w_o[:, r0:r0 + NH * HD].T * WSCALE             # [512, DM]
        wor = woT.reshape(2, 2, 128, 4, 512).transpose(2, 0, 3, 1, 4)
        wor = np.ascontiguousarray(wor)
        woTh = wor.astype(f8)
        woTl = (wor - woTh.astype(f32)).astype(f8)
        in_maps.append({
            "xh": pack_x(xTh), "xl": pack_x(xTl),
            "wh": pack_w(wTh), "wl": pack_w(wTl),
            "woh": woTh, "wol": woTl,
            "cosR": cosR, "sinR": sinR, "maskM": maskM, "maskA": maskA,
        })
    return in_maps


def kernel(hidden_states, cos, sin, w_qkv, w_o):
    nc = get_nc(legalized=True)
    in_maps = prep_core_inputs(hidden_states, cos, sin, w_qkv, w_o)
    res = run_bass_kernel_spmd(nc, in_maps, core_ids=list(range(8)))
    parts = [r["y"].astype(np.float32) / WSCALE for r in res.results]
    out = np.stack([
        parts[0] + parts[1] + parts[2] + parts[3],
        parts[4] + parts[5] + parts[6] + parts[7],
    ]).astype(np.float32)
    return out
